# revision 1
# baseline (speedup 1.0000x reference)
"""GemmaAttention on 8 Trainium2 NeuronCores, head-parallel (tensor parallel).

Shapes (hardcoded from the problem spec):
  hidden_states [2, 2048, 2048] f32
  attention_mask [2, 1, 2048, 2048] f32
  position_ids  [2, 2048] int64
  Wq [2048, 2048], Wk [256, 2048], Wv [256, 2048], Wo [2048, 2048] f32

Sharding: 8 query heads -> 8 cores. Each core computes its head's Q
projection, the (replicated) single-KV-head K/V projections, RoPE,
softmax attention, and a row-parallel o_proj partial. The o_proj
all-reduce is the host-side sum of the 8 partials; attn_weights are
gathered by stacking heads.
"""
import numpy as np
import jax
import jax.numpy as jnp
from functools import partial

B, S, H = 2, 2048, 2048
N_HEADS, N_KV, HEAD_DIM = 8, 1, 256
ROPE_THETA = 10000.0
N_CORES = 8

_compiled = None


def _rope_cos_sin_np(position_ids):
    inv_freq = 1.0 / (ROPE_THETA ** (np.arange(0, HEAD_DIM, 2, dtype=np.float32) / HEAD_DIM))
    freqs = position_ids.astype(np.float32)[:, :, None] * inv_freq[None, None, :]  # [B,S,hd/2]
    emb = np.concatenate((freqs, freqs), axis=-1)  # [B,S,hd]
    return np.cos(emb).astype(np.float32), np.sin(emb).astype(np.float32)


def _per_core(hs, mask, cos, sin, Wq_h, Wk, Wv, Wo_h):
    # hs [B,S,H]; Wq_h [hd,H]; Wk/Wv [hd,H]; Wo_h [H,hd]; cos/sin [B,S,hd]
    q = jnp.einsum('bsh,dh->bsd', hs, Wq_h)
    k = jnp.einsum('bsh,dh->bsd', hs, Wk)
    v = jnp.einsum('bsh,dh->bsd', hs, Wv)

    def rope(x):
        x1 = x[..., : HEAD_DIM // 2]
        rot = jnp.concatenate((-x1, x1), axis=-1)
        return x * cos + rot * sin

    q = rope(q)
    k = rope(k)
    scores = jnp.einsum('bqd,bkd->bqk', q, k) / jnp.sqrt(jnp.float32(HEAD_DIM))
    scores = scores + mask
    attn = jax.nn.softmax(scores, axis=-1)  # [B,S,S]
    attn_out = jnp.einsum('bqk,bkd->bqd', attn, v)  # [B,S,hd]
    out_part = jnp.einsum('bqd,hd->bqh', attn_out, Wo_h)  # [B,S,H]
    out = jax.lax.psum(out_part, axis_name='i')
    return out, attn


def _get_compiled():
    global _compiled
    if _compiled is None:
        _compiled = jax.pmap(_per_core, axis_name='i',
                             in_axes=(None, None, None, None, 0, None, None, 0))
    return _compiled


def kernel(hidden_states, attention_mask, position_ids, Wq, Wk, Wv, Wo):
    hs = np.asarray(hidden_states, np.float32)
    mask = np.asarray(attention_mask, np.float32)[:, 0]  # [B,S,S]
    cos, sin = _rope_cos_sin_np(np.asarray(position_ids))
    Wq_sh = np.asarray(Wq, np.float32).reshape(N_HEADS, HEAD_DIM, H)  # [8,hd,H]
    Wo_sh = np.ascontiguousarray(
        np.asarray(Wo, np.float32).reshape(H, N_HEADS, HEAD_DIM).transpose(1, 0, 2))  # [8,H,hd]

    fn = _get_compiled()
    out_rep, attn_sh = fn(hs, mask, cos, sin, Wq_sh,
                          np.asarray(Wk, np.float32), np.asarray(Wv, np.float32), Wo_sh)
    out = np.asarray(out_rep[0])                       # psum -> identical on all cores
    attn = np.asarray(attn_sh).transpose(1, 0, 2, 3)   # [8,B,S,S] -> [B,nH,S,S]
    return out, attn
